# revision 1
# baseline (speedup 1.0000x reference)
"""Causal GQA attention layer (B=2, S=2048, D=2048, 16 Q heads / 4 KV heads,
interleaved RoPE, causal softmax, output projection) on 8 TRN2 NeuronCores.

Sharding: core c -> (batch b = c//4, kv-group g = c%4). Each core owns 4 Q
heads + 1 KV head (tensor parallel over heads) for one batch element (data
parallel over batch). wq/wk/wv are column-sharded, wo is row-sharded; each
core emits a partial [S, D] output and the host sums the 4 partials per batch.

Device dataflow per core (all matmuls bf16 with fp32 PSUM accumulation):
  - QT/KT/VT projections computed directly in transposed [hd, s] layout from
    host-pretransposed x^T and W^T (contiguous DMAs, no device transposes).
    V and K run d-major with 4 parallel PSUM accumulators so PE consumes xt
    tiles in DMA-arrival order (fills the initial load ramp).
  - RoPE applied in transposed layout: pair-swap via a permutation matmul on
    PE, then cos/sin elementwise on DVE with host-precomputed [hd, s] tables.
  - Scores computed transposed, ST[k, q] = K_rot^T Q_rot, chunked [128, 512]
    with causal chunk skipping and valid-width restriction on diagonal
    chunks; exp on ScalarE (PSUM->SBUF, bf16, scale 1/sqrt(hd), no max
    subtraction -- |scores|*scale <= ~6 so exp is safe); triangular mask on
    the diagonal 128x128 block via gpsimd.affine_select. Scores run LOOKAHEAD
    chunks ahead of the exp->PV consumers (software pipeline) so the in-order
    PE stream never parks on the exp semaphore.
  - attnT[hd, q] accumulated in PSUM over k-tiles with V as weights (no P
    transposes anywhere). Softmax denominators accumulate on DVE in bf16 and
    reduce with a single ones-matmul per head; normalization is deferred to a
    per-q-chunk epilogue (reciprocal_approx_fast + gpsimd.partition_broadcast
    + one DVE multiply) so PSUM banks recycle immediately.
  - Output projection (row-parallel) interleaved into the attention phase as
    PE filler work: after each head of q-chunk qc, one output s-tile of
    qc-1 is emitted; the last q-chunk's tiles run in a deep-buffered tail.
"""

import math

import numpy as np
import ml_dtypes

import concourse.bass as bass
import concourse.tile as tile
from concourse import bacc
from concourse import mybir
from concourse import bass_utils

BF = ml_dtypes.bfloat16

B, S, D = 2, 2048, 2048
NH, NKV, HD = 16, 4, 128
P = 128
DT = D // P            # 16 contraction tiles
SCH = 512              # free-dim chunk
NSC = S // SCH         # 4
NST = S // P           # 16
HPG = NH // NKV        # 4 q heads per core
ROPE_BASE = 10000.0
SCALE = 1.0 / math.sqrt(HD)

TRACE = False
LAST_RESULTS = None


def _emit(nc, tc, aps):
    f32 = mybir.dt.float32
    bf16 = mybir.dt.bfloat16
    AF = mybir.ActivationFunctionType
    OP = mybir.AluOpType
    ctx_pools = []

    with tc.tile_pool(name="const", bufs=1) as cp:
        wvt = cp.tile([P, DT, HD], bf16)
        xt = cp.tile([P, DT, S], bf16)
        wkt = cp.tile([P, DT, HD], bf16)
        wqt = cp.tile([P, DT, HPG * HD], bf16)
        wot = cp.tile([P, HPG, D], bf16)
        cost = cp.tile([P, S], f32)
        sint = cp.tile([P, S], f32)
        perm = cp.tile([P, P], bf16)
        ident = cp.tile([P, P], bf16)
        ones1 = cp.tile([P, 1], bf16)
        # DMA triggers in consumption order (sc-major); HWDGE queue slots
        # round-robin in emission order across sync+scalar.
        engs = [nc.sync, nc.scalar]
        tq = []

        def dma(dst, src):
            tq.append((dst, src))

        dma(wvt[:], aps["wvt"][:])
        dma(xt[:, 0:4, 0:SCH], aps["xt"][:, 0:4, 0:SCH])
        dma(wqt[:, 0:4], aps["wqt"][:, 0:4])
        dma(wkt[:], aps["wkt"][:])
        dma(xt[:, 4:8, 0:SCH], aps["xt"][:, 4:8, 0:SCH])
        dma(wqt[:, 4:8], aps["wqt"][:, 4:8])
        dma(xt[:, 8:12, 0:SCH], aps["xt"][:, 8:12, 0:SCH])
        dma(wqt[:, 8:16], aps["wqt"][:, 8:16])
        dma(xt[:, 12:16, 0:SCH], aps["xt"][:, 12:16, 0:SCH])
        nc.gpsimd.dma_start(ident[:], aps["ident"][:])
        nc.gpsimd.dma_start(perm[:], aps["perm"][:])
        nc.gpsimd.dma_start(ones1[:], aps["ones1"][:])
        for sc in range(1, NSC):
            dma(xt[:, 0:8, sc * SCH:(sc + 1) * SCH],
                aps["xt"][:, 0:8, sc * SCH:(sc + 1) * SCH])
            dma(xt[:, 8:16, sc * SCH:(sc + 1) * SCH],
                aps["xt"][:, 8:16, sc * SCH:(sc + 1) * SCH])
            if sc == 1:
                dma(cost[:], aps["cost"][:])
                dma(sint[:], aps["sint"][:])
            if sc == 2:
                dma(wot[:, 0:2], aps["wot"][:, 0:2])
            if sc == 3:
                dma(wot[:, 2:4], aps["wot"][:, 2:4])
        for i, (dst, src_) in enumerate(tq):
            engs[i % 2].dma_start(dst, src_)

        # DVE warm-up: absorb the SWDGE semaphore tick for cost/sint into
        # DVE's vector clock so later tensor_tensor ops need only one fresh
        # wait (the TT encoding holds a single sync wait command).
        warm = cp.tile([1, 2], f32)
        nc.vector.tensor_copy(warm[:, 0:1], cost[0:1, 0:1])
        nc.vector.tensor_copy(warm[:, 1:2], sint[0:1, 0:1])
        # pre-swap the gpsimd ucode library during the DMA ramp so the
        # first affine_select/partition_broadcast in phase B doesn't pay
        # a ~5us mid-kernel library load
        warmg = cp.tile([P, 2], f32)
        nc.gpsimd.affine_select(
            out=warm[:, 0:1], in_=warm[:, 0:1], pattern=[[1, 1]],
            compare_op=OP.is_ge, fill=0.0, base=0, channel_multiplier=0)
        nc.gpsimd.partition_broadcast(warmg[:], warm[0:1, :])

        QROT = cp.tile([P, HPG, S], bf16)   # rotated Q^T per head
        KROT = cp.tile([P, S], bf16)        # rotated K^T
        VTS = cp.tile([P, S], bf16)         # V^T staging
        VN = cp.tile([P, NST, HD], bf16)    # V natural [s_in, s_tile, hd]
        ATTN = cp.tile([P, HPG, S], bf16)   # normalized attn^T per head
        qt3 = cp.tile([P, 5, SCH], bf16)    # last-chunk K/Q bf16 staging

        # ---- Phase A: projections + RoPE + V transpose, sc-major --------
        with tc.tile_pool(name="psA", bufs=1, space="PSUM") as psA, \
             tc.tile_pool(name="sbA", bufs=2) as sbA:

            def rope_pe(qt, psw):
                nc.tensor.matmul(psw[:], perm[:], qt, start=True, stop=True)

            def rope_dve(qt, psw, sc, dst_ap):
                t1 = sbA.tile([P, SCH], f32, tag="t1")
                nc.vector.tensor_tensor(
                    t1[:], psw[:], sint[:, sc * SCH:(sc + 1) * SCH], OP.mult)
                t2 = sbA.tile([P, SCH], f32, tag="t2")
                nc.vector.tensor_tensor(
                    t2[:], qt, cost[:, sc * SCH:(sc + 1) * SCH], OP.mult)
                nc.vector.tensor_tensor(dst_ap, t1[:], t2[:], OP.add)

            def proj_chunk(sc, inject):
                vps = psA.tile([P, SCH], f32, tag="v", name=f"vps{sc}")
                kps = psA.tile([P, SCH], f32, tag="k", name=f"kps{sc}")
                qps = [psA.tile([P, SCH], f32, tag=f"q{h}", name=f"qps{sc}_{h}")
                       for h in range(HPG)]
                xsl = xt[:, :, sc * SCH:(sc + 1) * SCH]
                for dt_ in range(DT):
                    if dt_ in inject:
                        inject[dt_]()
                    nc.tensor.matmul(vps[:], wvt[:, dt_, :], xsl[:, dt_],
                                     start=(dt_ == 0), stop=(dt_ == DT - 1))
                    for h in range(HPG):
                        nc.tensor.matmul(
                            qps[h][:], wqt[:, dt_, h * HD:(h + 1) * HD],
                            xsl[:, dt_],
                            start=(dt_ == 0), stop=(dt_ == DT - 1))
                    nc.tensor.matmul(kps[:], wkt[:, dt_, :], xsl[:, dt_],
                                     start=(dt_ == 0), stop=(dt_ == DT - 1))
                nc.scalar.copy(VTS[:, sc * SCH:(sc + 1) * SCH], vps[:])
                if sc == NSC - 1:
                    qtk = qt3[:, 0, :]
                    qtq = [qt3[:, 1 + h, :] for h in range(HPG)]
                else:
                    qtk = sbA.tile([P, SCH], bf16, tag="qt", bufs=10,
                                   name=f"qtk{sc}")[:]
                    qtq = [sbA.tile([P, SCH], bf16, tag="qt", bufs=10,
                                    name=f"qtq{sc}_{h}")[:]
                           for h in range(HPG)]
                nc.scalar.copy(qtk, kps[:])
                for h in range(HPG):
                    nc.scalar.copy(qtq[h], qps[h][:])
                return qtk, qtq

            def deferred(sc, qtk, qtq):
                inj = {}

                def at(dt_, fn):
                    prev = inj.get(dt_)
                    if prev is None:
                        inj[dt_] = fn
                    else:
                        def both(prev=prev, fn=fn):
                            prev()
                            fn()
                        inj[dt_] = both

                for j in range(4):
                    def vtrans(j=j):
                        ki = 4 * sc + j
                        pst = psA.tile([P, P], bf16, tag="tr", name=f"pst{ki}")
                        nc.tensor.transpose(
                            pst[:], VTS[:, ki * P:(ki + 1) * P], ident[:])
                        nc.scalar.copy(VN[:, ki, :], pst[:])
                    at(1 + j, vtrans)

                def ropek():
                    psw = psA.tile([P, SCH], f32, tag="w", name=f"pswk{sc}")
                    rope_pe(qtk, psw)
                    rope_dve(qtk, psw, sc, KROT[:, sc * SCH:(sc + 1) * SCH])
                at(6, ropek)
                for h in range(HPG):
                    def ropeq(h=h):
                        psw = psA.tile([P, SCH], f32, tag="w",
                                       name=f"pswq{sc}_{h}")
                        rope_pe(qtq[h], psw)
                        rope_dve(qtq[h], psw, sc,
                                 QROT[:, h, sc * SCH:(sc + 1) * SCH])
                    at(8 + 2 * h, ropeq)
                return inj

            prev = None
            for sc in range(NSC):
                inj = {} if prev is None else deferred(sc - 1, *prev)
                prev = proj_chunk(sc, inj)
            for j in range(4):
                ki = 4 * (NSC - 1) + j
                pst = psA.tile([P, P], bf16, tag="tr", name=f"pst{ki}")
                nc.tensor.transpose(
                    pst[:], VTS[:, ki * P:(ki + 1) * P], ident[:])
                nc.scalar.copy(VN[:, ki, :], pst[:])

        # ---- Phase B+C: attention (software-pipelined) with the output
        # projection interleaved to fill PE bubbles ----------------------
        # Loop qc outer / head inner. Denominators for the 4 heads of a qc
        # share one PSUM bank at 32-aligned partitions. Scores run LOOKAHEAD
        # chunks ahead of the exp->PV consumers so the in-order PE stream
        # never parks on the exp semaphore. After each head of qc, one
        # output-projection s-tile of qc-1 is emitted (its ATTN inputs are
        # long since normalized) as PE filler work.
        LOOK = 3
        with tc.tile_pool(name="psS", bufs=LOOK + 1, space="PSUM") as psS, \
             tc.tile_pool(name="psAV", bufs=2, space="PSUM") as psAV, \
             tc.tile_pool(name="psDN", bufs=1, space="PSUM") as psDN, \
             tc.tile_pool(name="psO", bufs=1, space="PSUM") as psO, \
             tc.tile_pool(name="sbB", bufs=LOOK + 4) as sbB, \
             tc.tile_pool(name="sbB2", bufs=5) as sbB2, \
             tc.tile_pool(name="sbN", bufs=2) as sbN, \
             tc.tile_pool(name="sbC", bufs=4) as sbC:

            def emit_c_group(st, oc, use_act=False):
                po = psO.tile([P, SCH], f32, tag="o")
                for ct in range(HPG):
                    nc.tensor.matmul(
                        po[:], ATTN[:, ct, st * P:(st + 1) * P],
                        wot[:, ct, oc * SCH:(oc + 1) * SCH],
                        start=(ct == 0), stop=(ct == HPG - 1))
                ob = sbC.tile([P, SCH], bf16, tag="ob")
                if use_act:
                    nc.scalar.copy(ob[:], po[:])
                else:
                    nc.vector.tensor_copy(ob[:], po[:])
                nc.sync.dma_start(
                    aps["out"][st * P:(st + 1) * P, oc * SCH:(oc + 1) * SCH], ob[:])

            sc3 = NSC - 1
            rope3 = list(range(5))
            groups = []

            def emit_rope3():
                j = rope3.pop(0)
                psw = psDN.tile([P, SCH], f32, tag="dn", name=f"psw3_{j}")
                nc.tensor.matmul(psw[:], perm[:], qt3[:, j, :],
                                 start=True, stop=True)
                t1 = sbN.tile([P, SCH], bf16, tag="t1", name=f"t13_{j}")
                nc.vector.tensor_tensor(
                    t1[:], psw[:], sint[:, sc3 * SCH:(sc3 + 1) * SCH], OP.mult)
                t2 = sbN.tile([P, SCH], bf16, tag="t2", name=f"t23_{j}")
                nc.vector.tensor_tensor(
                    t2[:], qt3[:, j, :], cost[:, sc3 * SCH:(sc3 + 1) * SCH],
                    OP.mult)
                dst = KROT[:, sc3 * SCH:(sc3 + 1) * SCH] if j == 0 \
                    else QROT[:, j - 1, sc3 * SCH:(sc3 + 1) * SCH]
                nc.vector.tensor_tensor(dst, t1[:], t2[:], OP.add)

            for qc in range(NSC):
                nki = 4 * qc + 4
                chunks = [(h, ki) for h in range(HPG) for ki in range(nki)]
                slots = {}
                accs = {}

                def emit_score(c, qc=qc):
                    h, ki = c
                    off = max(0, (ki - 4 * qc) * P)
                    n = SCH - off
                    pss = psS.tile([P, SCH], f32, tag="s")
                    nc.tensor.matmul(
                        pss[:, :n], KROT[:, ki * P:(ki + 1) * P],
                        QROT[:, h, qc * SCH + off: qc * SCH + off + n],
                        start=True, stop=True)
                    slots[c] = (pss, off, n)

                def emit_tail(c, qc=qc, nki=nki):
                    h, ki = c
                    pss, off, n = slots.pop(c)
                    if ki == 0:
                        accs[h] = (
                            psAV.tile([P, SCH], f32, tag="av", name=f"pav{qc}_{h}"),
                            sbB2.tile([P, SCH], bf16, tag="ps", name=f"pbs{qc}_{h}"),
                            sbB2.tile([P, SCH], bf16, tag="au", name=f"aun{qc}_{h}"))
                    pav, pbsum, aun = accs[h]
                    # ki==0: exp writes the running-sum tile directly (the
                    # first summand IS the exp output); PV reads it as moving
                    pb = pbsum if ki == 0 else sbB.tile([P, SCH], bf16, tag="p")
                    nc.scalar.activation(pb[:, :n], pss[:, :n], AF.Exp, scale=SCALE)
                    if ki >= 4 * qc:
                        nc.gpsimd.affine_select(
                            out=pb[:, :P], in_=pb[:, :P], pattern=[[1, P]],
                            compare_op=OP.is_ge, fill=0.0, base=0,
                            channel_multiplier=-1)
                    nc.tensor.matmul(
                        pav[:, off:], VN[:, ki, :], pb[:, :n],
                        start=(ki == 0), stop=(ki == nki - 1))
                    if ki > 0:
                        nc.vector.tensor_tensor(
                            pbsum[:, off:], pbsum[:, off:], pb[:, :n], OP.add)
                    if ki == nki - 1:
                        # unnormalized attn out of PSUM fast (frees the bank);
                        # normalization is deferred to the per-qc epilogue
                        nc.scalar.copy(aun[:], pav[:])
                        if qc > 0:
                            st = (qc - 1) * 4 + h
                            for oc in range(NSC):
                                groups.append((st, oc))

                for i in range(len(chunks) + LOOK):
                    if i < len(chunks):
                        emit_score(chunks[i])
                    if i >= LOOK:
                        emit_tail(chunks[i - LOOK])
                        if groups:
                            st_, oc_ = groups.pop(0)
                            emit_c_group(st_, oc_, use_act=(oc_ == 3))
                        if qc == 0 and (i - LOOK) % 3 == 2 and rope3:
                            emit_rope3()

                while groups:
                    st_, oc_ = groups.pop(0)
                    emit_c_group(st_, oc_, use_act=(oc_ == 3))
                for h in range(HPG):
                    _, pbsum, aun = accs[h]
                    pdn = psDN.tile([1, SCH], f32, tag="dn")
                    nc.tensor.matmul(pdn[:], ones1[:], pbsum[:],
                                     start=True, stop=True)
                    rc = sbN.tile([1, SCH], f32, tag="rc")
                    nc.vector.reciprocal_approx_fast(out=rc[:], in_=pdn[:])
                    bc = sbN.tile([P, SCH], f32, tag="bc")
                    nc.gpsimd.partition_broadcast(bc[:], rc[:])
                    nc.vector.tensor_tensor(
                        ATTN[:, h, qc * SCH:(qc + 1) * SCH], aun[:], bc[:],
                        OP.mult)

        # ---- tail: last q-chunk's output tiles with deep PSUM buffering
        with tc.tile_pool(name="psO2", bufs=6, space="PSUM") as psO2, \
             tc.tile_pool(name="sbC2", bufs=3) as sbC2:
            for st in range(3 * 4, NST):
                ob = sbC2.tile([P, D], bf16, tag="ob2", name=f"obt{st}")
                for oc in range(NSC):
                    po = psO2.tile([P, SCH], f32, tag="o2")
                    for ct in range(HPG):
                        nc.tensor.matmul(
                            po[:], ATTN[:, ct, st * P:(st + 1) * P],
                            wot[:, ct, oc * SCH:(oc + 1) * SCH],
                            start=(ct == 0), stop=(ct == HPG - 1))
                    if oc % 2 == 0:
                        nc.scalar.copy(ob[:, oc * SCH:(oc + 1) * SCH], po[:])
                    else:
                        nc.vector.tensor_copy(ob[:, oc * SCH:(oc + 1) * SCH], po[:])
                eng = nc.sync if st % 2 == 0 else nc.scalar
                eng.dma_start(aps["out"][st * P:(st + 1) * P, :], ob[:])



def _build_program():
    f32 = mybir.dt.float32
    bf16 = mybir.dt.bfloat16
    nc = bacc.Bacc("TRN2", debug=False, target_bir_lowering=False)
    aps = {
        "xt": nc.dram_tensor("xt", [P, DT, S], bf16, kind="ExternalInput").ap(),
        "wqt": nc.dram_tensor("wqt", [P, DT, HPG * HD], bf16, kind="ExternalInput").ap(),
        "wkt": nc.dram_tensor("wkt", [P, DT, HD], bf16, kind="ExternalInput").ap(),
        "wvt": nc.dram_tensor("wvt", [P, DT, HD], bf16, kind="ExternalInput").ap(),
        "wot": nc.dram_tensor("wot", [P, HPG, D], bf16, kind="ExternalInput").ap(),
        "cost": nc.dram_tensor("cost", [P, S], f32, kind="ExternalInput").ap(),
        "sint": nc.dram_tensor("sint", [P, S], f32, kind="ExternalInput").ap(),
        "perm": nc.dram_tensor("perm", [P, P], bf16, kind="ExternalInput").ap(),
        "ident": nc.dram_tensor("ident", [P, P], bf16, kind="ExternalInput").ap(),
        "ones1": nc.dram_tensor("ones1", [P, 1], bf16, kind="ExternalInput").ap(),
        "out": nc.dram_tensor("out", [S, D], bf16, kind="ExternalOutput").ap(),
    }
    with tile.TileContext(nc) as tc:
        _emit(nc, tc, aps)
    nc.compile()
    return nc


def _tables():
    theta = 1.0 / (ROPE_BASE ** (np.arange(0, HD, 2, dtype=np.float64) / HD))
    ang = np.outer(np.arange(S, dtype=np.float64), theta)      # [S, 64]
    cosT = np.repeat(np.cos(ang).T, 2, axis=0).astype(np.float32)  # [128, S]
    sinT = np.repeat(np.sin(ang).T, 2, axis=0)
    sign = np.where(np.arange(HD) % 2 == 0, -1.0, 1.0)[:, None]
    sinsT = (sinT * sign).astype(np.float32)
    perm = np.zeros((P, P), dtype=BF)
    idx = np.arange(P)
    perm[idx, idx ^ 1] = 1
    ident = np.eye(P, dtype=np.float32).astype(BF)
    ones1 = np.ones((P, 1), dtype=BF)
    return cosT, sinsT, perm, ident, ones1


def _in_maps(x, wq, wk, wv, wo):
    cosT, sinsT, perm, ident, ones1 = _tables()
    maps = []
    for c in range(8):
        b, g = divmod(c, NKV)
        xt = x[b].T.reshape(DT, P, S).transpose(1, 0, 2).astype(BF)
        wqg = wq[g * HPG * HD:(g + 1) * HPG * HD]
        wqt = wqg.T.reshape(DT, P, HPG * HD).transpose(1, 0, 2).astype(BF)
        wkt = wk[g * HD:(g + 1) * HD].T.reshape(DT, P, HD).transpose(1, 0, 2).astype(BF)
        wvt = wv[g * HD:(g + 1) * HD].T.reshape(DT, P, HD).transpose(1, 0, 2).astype(BF)
        wog = wo[:, g * HPG * HD:(g + 1) * HPG * HD]
        wot = wog.T.reshape(HPG, P, D).transpose(1, 0, 2).astype(BF)
        maps.append({
            "xt": np.ascontiguousarray(xt),
            "wqt": np.ascontiguousarray(wqt),
            "wkt": np.ascontiguousarray(wkt),
            "wvt": np.ascontiguousarray(wvt),
            "wot": np.ascontiguousarray(wot),
            "cost": cosT, "sint": sinsT,
            "perm": perm, "ident": ident, "ones1": ones1,
        })
    return maps


_PROGRAM = None


def kernel(x, wq, wk, wv, wo):
    global _PROGRAM, LAST_RESULTS
    x = np.asarray(x, dtype=np.float32)
    wq = np.asarray(wq, dtype=np.float32)
    wk = np.asarray(wk, dtype=np.float32)
    wv = np.asarray(wv, dtype=np.float32)
    wo = np.asarray(wo, dtype=np.float32)
    if _PROGRAM is None:
        _PROGRAM = _build_program()
    res = bass_utils.run_bass_kernel_spmd(
        _PROGRAM, _in_maps(x, wq, wk, wv, wo),
        core_ids=list(range(8)), trace=TRACE)
    LAST_RESULTS = res
    out = np.zeros((B, S, D), np.float32)
    for c in range(8):
        out[c // NKV] += np.asarray(res.results[c]["out"], dtype=np.float32)
    return out



# revision 5
# speedup vs baseline: 1.1484x; 1.1484x over previous
"""Causal GQA attention layer (B=2, S=2048, D=2048, 16 Q heads / 4 KV heads,
interleaved RoPE, causal softmax, output projection) on 8 TRN2 NeuronCores.

Sharding: core c -> (batch b = c//4, kv-group g = c%4). Each core owns 4 Q
heads + 1 KV head (tensor parallel over heads) for one batch element (data
parallel over batch). wq/wk/wv are column-sharded, wo is row-sharded; each
core emits a partial [S, D] output and the host sums the 4 partials per batch.

Device dataflow per core (all matmuls bf16 with fp32 PSUM accumulation):
  - QT/KT/VT projections computed directly in transposed [hd, s] layout from
    host-pretransposed x^T and W^T (contiguous DMAs, no device transposes).
    V and K run d-major with 4 parallel PSUM accumulators so PE consumes xt
    tiles in DMA-arrival order (fills the initial load ramp).
  - RoPE applied in transposed layout: pair-swap via a permutation matmul on
    PE, then cos/sin elementwise on DVE with host-precomputed [hd, s] tables.
  - Scores computed transposed, ST[k, q] = K_rot^T Q_rot, chunked [128, 512]
    with causal chunk skipping and valid-width restriction on diagonal
    chunks; exp on ScalarE (PSUM->SBUF, bf16, scale 1/sqrt(hd), no max
    subtraction -- |scores|*scale <= ~6 so exp is safe); triangular mask on
    the diagonal 128x128 block via gpsimd.affine_select. Scores run LOOKAHEAD
    chunks ahead of the exp->PV consumers (software pipeline) so the in-order
    PE stream never parks on the exp semaphore.
  - attnT[hd, q] accumulated in PSUM over k-tiles with V as weights (no P
    transposes anywhere). Softmax denominators accumulate on DVE in bf16 and
    reduce with a single ones-matmul per head; normalization is deferred to a
    per-q-chunk epilogue (reciprocal_approx_fast + gpsimd.partition_broadcast
    + one DVE multiply) so PSUM banks recycle immediately.
  - Output projection (row-parallel) interleaved into the attention phase as
    PE filler work: after each head of q-chunk qc, one output s-tile of
    qc-1 is emitted; the last q-chunk's tiles run in a deep-buffered tail.
"""

import math

import numpy as np
import ml_dtypes

import concourse.bass as bass
import concourse.tile as tile
from concourse import bacc
from concourse import mybir
from concourse import bass_utils

BF = ml_dtypes.bfloat16

B, S, D = 2, 2048, 2048
NH, NKV, HD = 16, 4, 128
P = 128
DT = D // P            # 16 contraction tiles
SCH = 512              # free-dim chunk
NSC = S // SCH         # 4
NST = S // P           # 16
HPG = NH // NKV        # 4 q heads per core
ROPE_BASE = 10000.0
SCALE = 1.0 / math.sqrt(HD)

TRACE = False
LAST_RESULTS = None


NWARM = 24


def _emit(nc, tc, aps):
    f32 = mybir.dt.float32
    bf16 = mybir.dt.bfloat16
    AF = mybir.ActivationFunctionType
    OP = mybir.AluOpType
    ctx_pools = []

    # ---- Phase 0: PE warmup ----------------------------------------
    # Garbage matmuls on a zeroed tile, emitted before any DMA-dependent
    # work: the PE is busy from the post-barrier instant (~3.5us), so the
    # HAM clock-gate warms to 8/8 during the DMA ramp (instead of ~24us)
    # and the first real matmuls run at full clock with no initial idle.
    with tc.tile_pool(name="warm0", bufs=1) as wp, \
         tc.tile_pool(name="psW", bufs=1, space="PSUM") as psW:
        wz = wp.tile([P, SCH], bf16)
        nc.vector.memset(wz[:], 0.0)
        pw = psW.tile([P, SCH], f32, tag="w")
        for _ in range(NWARM):
            nc.tensor.matmul(pw[:], wz[:, 0:P], wz[:], start=True, stop=True)

    with tc.tile_pool(name="const", bufs=1) as cp:
        wvt = cp.tile([P, DT, HD], bf16)
        xt = cp.tile([P, DT, S], bf16)
        wkt = cp.tile([P, DT, HD], bf16)
        wqt = cp.tile([P, DT, HPG * HD], bf16)
        wot = cp.tile([P, HPG, D], bf16)
        cost = cp.tile([P, S], f32)
        sint = cp.tile([P, S], f32)
        perm = cp.tile([P, P], bf16)
        ident = cp.tile([P, P], bf16)
        ones1 = cp.tile([P, 1], bf16)
        # DMA triggers in consumption order (sc-major); HWDGE queue slots
        # round-robin in emission order across sync+scalar.
        engs = [nc.sync, nc.scalar]
        tq = []

        def dma(dst, src):
            tq.append((dst, src))

        dma(wvt[:], aps["wvt"][:])
        dma(xt[:, 0:4, 0:SCH], aps["xt"][:, 0:4, 0:SCH])
        dma(wqt[:, 0:4], aps["wqt"][:, 0:4])
        dma(wkt[:], aps["wkt"][:])
        dma(xt[:, 4:8, 0:SCH], aps["xt"][:, 4:8, 0:SCH])
        dma(wqt[:, 4:8], aps["wqt"][:, 4:8])
        dma(xt[:, 8:12, 0:SCH], aps["xt"][:, 8:12, 0:SCH])
        dma(wqt[:, 8:16], aps["wqt"][:, 8:16])
        dma(xt[:, 12:16, 0:SCH], aps["xt"][:, 12:16, 0:SCH])
        nc.gpsimd.dma_start(ident[:], aps["ident"][:])
        nc.gpsimd.dma_start(perm[:], aps["perm"][:])
        nc.gpsimd.dma_start(ones1[:], aps["ones1"][:])
        for sc in range(1, NSC):
            dma(xt[:, 0:8, sc * SCH:(sc + 1) * SCH],
                aps["xt"][:, 0:8, sc * SCH:(sc + 1) * SCH])
            dma(xt[:, 8:16, sc * SCH:(sc + 1) * SCH],
                aps["xt"][:, 8:16, sc * SCH:(sc + 1) * SCH])
            if sc == 1:
                dma(cost[:], aps["cost"][:])
                dma(sint[:], aps["sint"][:])
            if sc == 2:
                dma(wot[:, 0:2], aps["wot"][:, 0:2])
            if sc == 3:
                dma(wot[:, 2:4], aps["wot"][:, 2:4])
        for i, (dst, src_) in enumerate(tq):
            engs[i % 2].dma_start(dst, src_)

        # DVE warm-up: absorb the SWDGE semaphore tick for cost/sint into
        # DVE's vector clock so later tensor_tensor ops need only one fresh
        # wait (the TT encoding holds a single sync wait command).
        warm = cp.tile([1, 2], f32)
        nc.vector.tensor_copy(warm[:, 0:1], cost[0:1, 0:1])
        nc.vector.tensor_copy(warm[:, 1:2], sint[0:1, 0:1])
        # pre-swap the gpsimd ucode library during the DMA ramp so the
        # first affine_select/partition_broadcast in phase B doesn't pay
        # a ~5us mid-kernel library load
        warmg = cp.tile([P, 2], f32)
        nc.gpsimd.affine_select(
            out=warm[:, 0:1], in_=warm[:, 0:1], pattern=[[1, 1]],
            compare_op=OP.is_ge, fill=0.0, base=0, channel_multiplier=0)
        nc.gpsimd.partition_broadcast(warmg[:], warm[0:1, :])

        QROT = cp.tile([P, HPG, S], bf16)   # rotated Q^T per head
        KROT = cp.tile([P, S], bf16)        # rotated K^T
        VTS = cp.tile([P, S], bf16)         # V^T staging
        VN = cp.tile([P, NST, HD], bf16)    # V natural [s_in, s_tile, hd]
        ATTN = cp.tile([P, HPG, S], bf16)   # normalized attn^T per head
        qt3 = cp.tile([P, 5, SCH], bf16)    # last-chunk K/Q bf16 staging

        # ---- Phase A: projections + RoPE + V transpose, sc-major --------
        with tc.tile_pool(name="psA", bufs=1, space="PSUM") as psA, \
             tc.tile_pool(name="sbA", bufs=2) as sbA:

            def rope_pe(qt, psw):
                nc.tensor.matmul(psw[:], perm[:], qt, start=True, stop=True)

            def rope_dve(qt, psw, sc, dst_ap):
                t1 = sbA.tile([P, SCH], f32, tag="t1")
                nc.vector.tensor_tensor(
                    t1[:], psw[:], sint[:, sc * SCH:(sc + 1) * SCH], OP.mult)
                t2 = sbA.tile([P, SCH], f32, tag="t2")
                nc.vector.tensor_tensor(
                    t2[:], qt, cost[:, sc * SCH:(sc + 1) * SCH], OP.mult)
                nc.vector.tensor_tensor(dst_ap, t1[:], t2[:], OP.add)

            def proj_chunk(sc, inject):
                vps = psA.tile([P, SCH], f32, tag="v", name=f"vps{sc}")
                kps = psA.tile([P, SCH], f32, tag="k", name=f"kps{sc}")
                qps = [psA.tile([P, SCH], f32, tag=f"q{h}", name=f"qps{sc}_{h}")
                       for h in range(HPG)]
                xsl = xt[:, :, sc * SCH:(sc + 1) * SCH]
                for dt_ in range(DT):
                    if dt_ in inject:
                        inject[dt_]()
                    nc.tensor.matmul(vps[:], wvt[:, dt_, :], xsl[:, dt_],
                                     start=(dt_ == 0), stop=(dt_ == DT - 1))
                    for h in range(HPG):
                        nc.tensor.matmul(
                            qps[h][:], wqt[:, dt_, h * HD:(h + 1) * HD],
                            xsl[:, dt_],
                            start=(dt_ == 0), stop=(dt_ == DT - 1))
                    nc.tensor.matmul(kps[:], wkt[:, dt_, :], xsl[:, dt_],
                                     start=(dt_ == 0), stop=(dt_ == DT - 1))
                nc.scalar.copy(VTS[:, sc * SCH:(sc + 1) * SCH], vps[:])
                if sc == NSC - 1:
                    qtk = qt3[:, 0, :]
                    qtq = [qt3[:, 1 + h, :] for h in range(HPG)]
                else:
                    qtk = sbA.tile([P, SCH], bf16, tag="qt", bufs=10,
                                   name=f"qtk{sc}")[:]
                    qtq = [sbA.tile([P, SCH], bf16, tag="qt", bufs=10,
                                    name=f"qtq{sc}_{h}")[:]
                           for h in range(HPG)]
                nc.scalar.copy(qtk, kps[:])
                for h in range(HPG):
                    nc.scalar.copy(qtq[h], qps[h][:])
                return qtk, qtq

            def deferred(sc, qtk, qtq):
                inj = {}

                def at(dt_, fn):
                    prev = inj.get(dt_)
                    if prev is None:
                        inj[dt_] = fn
                    else:
                        def both(prev=prev, fn=fn):
                            prev()
                            fn()
                        inj[dt_] = both

                for j in range(4):
                    def vtrans(j=j):
                        ki = 4 * sc + j
                        pst = psA.tile([P, P], bf16, tag="tr", name=f"pst{ki}")
                        nc.tensor.transpose(
                            pst[:], VTS[:, ki * P:(ki + 1) * P], ident[:])
                        nc.scalar.copy(VN[:, ki, :], pst[:])
                    at(1 + j, vtrans)

                def ropek():
                    psw = psA.tile([P, SCH], f32, tag="w", name=f"pswk{sc}")
                    rope_pe(qtk, psw)
                    rope_dve(qtk, psw, sc, KROT[:, sc * SCH:(sc + 1) * SCH])
                at(6, ropek)
                for h in range(HPG):
                    def ropeq(h=h):
                        psw = psA.tile([P, SCH], f32, tag="w",
                                       name=f"pswq{sc}_{h}")
                        rope_pe(qtq[h], psw)
                        rope_dve(qtq[h], psw, sc,
                                 QROT[:, h, sc * SCH:(sc + 1) * SCH])
                    at(8 + 2 * h, ropeq)
                return inj

            prev = None
            for sc in range(NSC):
                inj = {} if prev is None else deferred(sc - 1, *prev)
                prev = proj_chunk(sc, inj)
            for j in range(4):
                ki = 4 * (NSC - 1) + j
                pst = psA.tile([P, P], bf16, tag="tr", name=f"pst{ki}")
                nc.tensor.transpose(
                    pst[:], VTS[:, ki * P:(ki + 1) * P], ident[:])
                nc.scalar.copy(VN[:, ki, :], pst[:])

        # ---- Phase B+C: attention (software-pipelined) with the output
        # projection interleaved to fill PE bubbles ----------------------
        # Loop qc outer / head inner. Denominators for the 4 heads of a qc
        # share one PSUM bank at 32-aligned partitions. Scores run LOOKAHEAD
        # chunks ahead of the exp->PV consumers so the in-order PE stream
        # never parks on the exp semaphore. After each head of qc, one
        # output-projection s-tile of qc-1 is emitted (its ATTN inputs are
        # long since normalized) as PE filler work.
        LOOK = 3
        with tc.tile_pool(name="psS", bufs=LOOK + 1, space="PSUM") as psS, \
             tc.tile_pool(name="psAV", bufs=2, space="PSUM") as psAV, \
             tc.tile_pool(name="psDN", bufs=1, space="PSUM") as psDN, \
             tc.tile_pool(name="psO", bufs=1, space="PSUM") as psO, \
             tc.tile_pool(name="sbB", bufs=LOOK + 4) as sbB, \
             tc.tile_pool(name="sbB2", bufs=5) as sbB2, \
             tc.tile_pool(name="sbN", bufs=2) as sbN, \
             tc.tile_pool(name="sbC", bufs=4) as sbC:

            def emit_c_group(st, oc, use_act=False):
                po = psO.tile([P, SCH], f32, tag="o")
                for ct in range(HPG):
                    nc.tensor.matmul(
                        po[:], ATTN[:, ct, st * P:(st + 1) * P],
                        wot[:, ct, oc * SCH:(oc + 1) * SCH],
                        start=(ct == 0), stop=(ct == HPG - 1))
                ob = sbC.tile([P, SCH], bf16, tag="ob")
                if use_act:
                    nc.scalar.copy(ob[:], po[:])
                else:
                    nc.vector.tensor_copy(ob[:], po[:])
                nc.sync.dma_start(
                    aps["out"][st * P:(st + 1) * P, oc * SCH:(oc + 1) * SCH], ob[:])

            sc3 = NSC - 1
            rope3 = list(range(5))
            groups = []
            epi_q = []      # deferred per-head epilogues, popped 1/step
            carry_epi = []  # h=3 epilogue carried into the next qc's loop

            def make_epi(qc, h, pbsum, aun):
                def epi():
                    pdn = psDN.tile([1, SCH], f32, tag="dn")
                    nc.tensor.matmul(pdn[:], ones1[:], pbsum[:],
                                     start=True, stop=True)
                    rc = sbN.tile([1, SCH], f32, tag="rc")
                    nc.vector.reciprocal_approx_fast(out=rc[:], in_=pdn[:])
                    bc = sbN.tile([P, SCH], f32, tag="bc")
                    nc.gpsimd.partition_broadcast(bc[:], rc[:])
                    nc.vector.tensor_tensor(
                        ATTN[:, h, qc * SCH:(qc + 1) * SCH], aun[:], bc[:],
                        OP.mult)
                return epi

            def emit_rope3():
                j = rope3.pop(0)
                psw = psDN.tile([P, SCH], f32, tag="dn", name=f"psw3_{j}")
                nc.tensor.matmul(psw[:], perm[:], qt3[:, j, :],
                                 start=True, stop=True)
                t1 = sbN.tile([P, SCH], bf16, tag="t1", name=f"t13_{j}")
                nc.vector.tensor_tensor(
                    t1[:], psw[:], sint[:, sc3 * SCH:(sc3 + 1) * SCH], OP.mult)
                t2 = sbN.tile([P, SCH], bf16, tag="t2", name=f"t23_{j}")
                nc.vector.tensor_tensor(
                    t2[:], qt3[:, j, :], cost[:, sc3 * SCH:(sc3 + 1) * SCH],
                    OP.mult)
                dst = KROT[:, sc3 * SCH:(sc3 + 1) * SCH] if j == 0 \
                    else QROT[:, j - 1, sc3 * SCH:(sc3 + 1) * SCH]
                nc.vector.tensor_tensor(dst, t1[:], t2[:], OP.add)

            for qc in range(NSC):
                nki = 4 * qc + 4
                chunks = [(h, ki) for h in range(HPG) for ki in range(nki)]
                slots = {}
                accs = {}

                def emit_score(c, qc=qc):
                    h, ki = c
                    off = max(0, (ki - 4 * qc) * P)
                    n = SCH - off
                    pss = psS.tile([P, SCH], f32, tag="s")
                    nc.tensor.matmul(
                        pss[:, :n], KROT[:, ki * P:(ki + 1) * P],
                        QROT[:, h, qc * SCH + off: qc * SCH + off + n],
                        start=True, stop=True)
                    slots[c] = (pss, off, n)

                def emit_tail(c, qc=qc, nki=nki):
                    h, ki = c
                    pss, off, n = slots.pop(c)
                    if ki == 0:
                        accs[h] = (
                            psAV.tile([P, SCH], f32, tag="av", name=f"pav{qc}_{h}"),
                            sbB2.tile([P, SCH], bf16, tag="ps", name=f"pbs{qc}_{h}"),
                            sbB2.tile([P, SCH], bf16, tag="au", name=f"aun{qc}_{h}"))
                    pav, pbsum, aun = accs[h]
                    # ki==0: exp writes the running-sum tile directly (the
                    # first summand IS the exp output); PV reads it as moving
                    pb = pbsum if ki == 0 else sbB.tile([P, SCH], bf16, tag="p")
                    nc.scalar.activation(pb[:, :n], pss[:, :n], AF.Exp, scale=SCALE)
                    if ki >= 4 * qc:
                        nc.gpsimd.affine_select(
                            out=pb[:, :P], in_=pb[:, :P], pattern=[[1, P]],
                            compare_op=OP.is_ge, fill=0.0, base=0,
                            channel_multiplier=-1)
                    nc.tensor.matmul(
                        pav[:, off:], VN[:, ki, :], pb[:, :n],
                        start=(ki == 0), stop=(ki == nki - 1))
                    if ki > 0:
                        nc.vector.tensor_tensor(
                            pbsum[:, off:], pbsum[:, off:], pb[:, :n], OP.add)
                    if ki == nki - 1:
                        # unnormalized attn out of PSUM fast (frees the bank);
                        # normalization runs as a deferred per-head epilogue
                        nc.scalar.copy(aun[:], pav[:])
                        if h < HPG - 1 or qc == NSC - 1:
                            # heads 0-2 (and qc3's h3 via the post-flush
                            # path below) normalize inside this qc's loop
                            if h < HPG - 1:
                                epi_q.append(make_epi(qc, h, pbsum, aun))
                        else:
                            # h=3 of qc<3: normalize early in qc+1's loop so
                            # the PE never parks on the exp->pbsum chain at
                            # the qc boundary (consumers have ~nki steps of
                            # slack before the first out-proj group pops)
                            carry_epi.append(make_epi(qc, h, pbsum, aun))
                        if qc > 0:
                            st = (qc - 1) * 4 + h
                            for oc in range(NSC):
                                groups.append((st, oc))

                epi_q.extend(carry_epi)
                carry_epi = []
                for i in range(len(chunks) + LOOK):
                    if i < len(chunks):
                        emit_score(chunks[i])
                    if i >= LOOK:
                        emit_tail(chunks[i - LOOK])
                        if groups:
                            st_, oc_ = groups.pop(0)
                            emit_c_group(st_, oc_, use_act=(oc_ == 3))
                        if epi_q:
                            epi_q.pop(0)()
                        if qc == 0 and (i - LOOK) % 3 == 2 and rope3:
                            emit_rope3()

                while groups:
                    st_, oc_ = groups.pop(0)
                    emit_c_group(st_, oc_, use_act=False)
                while epi_q:
                    epi_q.pop(0)()
                if qc == NSC - 1:
                    # qc3 h=3: the only epilogue on the tail critical path.
                    # Normalize in 4 per-st pieces so the first tail out-proj
                    # tile starts after a [128,128] multiply, not the full
                    # 512-wide one.
                    _, pbsum, aun = accs[HPG - 1]
                    pdn = psDN.tile([1, SCH], f32, tag="dn")
                    nc.tensor.matmul(pdn[:], ones1[:], pbsum[:],
                                     start=True, stop=True)
                    rc = sbN.tile([1, SCH], f32, tag="rc")
                    nc.vector.reciprocal_approx_fast(out=rc[:], in_=pdn[:])
                    bc = sbN.tile([P, SCH], f32, tag="bc")
                    nc.gpsimd.partition_broadcast(bc[:], rc[:])
                    for stp in range(4):
                        nc.vector.tensor_tensor(
                            ATTN[:, HPG - 1,
                                 qc * SCH + stp * P: qc * SCH + (stp + 1) * P],
                            aun[:, stp * P:(stp + 1) * P],
                            bc[:, stp * P:(stp + 1) * P], OP.mult)

        # ---- tail: last q-chunk's output tiles with deep PSUM buffering.
        # DMA fires per [128,1024] half as soon as its two oc-chunks are
        # copied, round-robin across sync/scalar/gpsimd queues, so the last
        # transfer is small and the drain overlaps the remaining compute.
        with tc.tile_pool(name="psO2", bufs=6, space="PSUM") as psO2, \
             tc.tile_pool(name="sbC2", bufs=3) as sbC2:
            dma_rr = [nc.sync, nc.scalar, nc.gpsimd]
            nrr = 0
            for st in range(3 * 4, NST):
                ob = sbC2.tile([P, D], bf16, tag="ob2", name=f"obt{st}")
                for oc in range(NSC):
                    po = psO2.tile([P, SCH], f32, tag="o2")
                    for ct in range(HPG):
                        nc.tensor.matmul(
                            po[:], ATTN[:, ct, st * P:(st + 1) * P],
                            wot[:, ct, oc * SCH:(oc + 1) * SCH],
                            start=(ct == 0), stop=(ct == HPG - 1))
                    if oc % 2 == 0:
                        nc.scalar.copy(ob[:, oc * SCH:(oc + 1) * SCH], po[:])
                    else:
                        nc.vector.tensor_copy(ob[:, oc * SCH:(oc + 1) * SCH], po[:])
                    if oc % 2 == 1:
                        half = (oc // 2) * 2 * SCH
                        dma_rr[nrr % 3].dma_start(
                            aps["out"][st * P:(st + 1) * P, half:half + 2 * SCH],
                            ob[:, half:half + 2 * SCH])
                        nrr += 1



def _build_program():
    f32 = mybir.dt.float32
    bf16 = mybir.dt.bfloat16
    nc = bacc.Bacc("TRN2", debug=False, target_bir_lowering=False)
    aps = {
        "xt": nc.dram_tensor("xt", [P, DT, S], bf16, kind="ExternalInput").ap(),
        "wqt": nc.dram_tensor("wqt", [P, DT, HPG * HD], bf16, kind="ExternalInput").ap(),
        "wkt": nc.dram_tensor("wkt", [P, DT, HD], bf16, kind="ExternalInput").ap(),
        "wvt": nc.dram_tensor("wvt", [P, DT, HD], bf16, kind="ExternalInput").ap(),
        "wot": nc.dram_tensor("wot", [P, HPG, D], bf16, kind="ExternalInput").ap(),
        "cost": nc.dram_tensor("cost", [P, S], f32, kind="ExternalInput").ap(),
        "sint": nc.dram_tensor("sint", [P, S], f32, kind="ExternalInput").ap(),
        "perm": nc.dram_tensor("perm", [P, P], bf16, kind="ExternalInput").ap(),
        "ident": nc.dram_tensor("ident", [P, P], bf16, kind="ExternalInput").ap(),
        "ones1": nc.dram_tensor("ones1", [P, 1], bf16, kind="ExternalInput").ap(),
        "out": nc.dram_tensor("out", [S, D], bf16, kind="ExternalOutput").ap(),
    }
    with tile.TileContext(nc) as tc:
        _emit(nc, tc, aps)
    nc.compile()
    return nc


def _tables():
    theta = 1.0 / (ROPE_BASE ** (np.arange(0, HD, 2, dtype=np.float64) / HD))
    ang = np.outer(np.arange(S, dtype=np.float64), theta)      # [S, 64]
    cosT = np.repeat(np.cos(ang).T, 2, axis=0).astype(np.float32)  # [128, S]
    sinT = np.repeat(np.sin(ang).T, 2, axis=0)
    sign = np.where(np.arange(HD) % 2 == 0, -1.0, 1.0)[:, None]
    sinsT = (sinT * sign).astype(np.float32)
    perm = np.zeros((P, P), dtype=BF)
    idx = np.arange(P)
    perm[idx, idx ^ 1] = 1
    ident = np.eye(P, dtype=np.float32).astype(BF)
    ones1 = np.ones((P, 1), dtype=BF)
    return cosT, sinsT, perm, ident, ones1


def _in_maps(x, wq, wk, wv, wo):
    cosT, sinsT, perm, ident, ones1 = _tables()
    maps = []
    for c in range(8):
        b, g = divmod(c, NKV)
        xt = x[b].T.reshape(DT, P, S).transpose(1, 0, 2).astype(BF)
        wqg = wq[g * HPG * HD:(g + 1) * HPG * HD]
        wqt = wqg.T.reshape(DT, P, HPG * HD).transpose(1, 0, 2).astype(BF)
        wkt = wk[g * HD:(g + 1) * HD].T.reshape(DT, P, HD).transpose(1, 0, 2).astype(BF)
        wvt = wv[g * HD:(g + 1) * HD].T.reshape(DT, P, HD).transpose(1, 0, 2).astype(BF)
        wog = wo[:, g * HPG * HD:(g + 1) * HPG * HD]
        wot = wog.T.reshape(HPG, P, D).transpose(1, 0, 2).astype(BF)
        maps.append({
            "xt": np.ascontiguousarray(xt),
            "wqt": np.ascontiguousarray(wqt),
            "wkt": np.ascontiguousarray(wkt),
            "wvt": np.ascontiguousarray(wvt),
            "wot": np.ascontiguousarray(wot),
            "cost": cosT, "sint": sinsT,
            "perm": perm, "ident": ident, "ones1": ones1,
        })
    return maps


_PROGRAM = None


def kernel(x, wq, wk, wv, wo):
    global _PROGRAM, LAST_RESULTS
    x = np.asarray(x, dtype=np.float32)
    wq = np.asarray(wq, dtype=np.float32)
    wk = np.asarray(wk, dtype=np.float32)
    wv = np.asarray(wv, dtype=np.float32)
    wo = np.asarray(wo, dtype=np.float32)
    if _PROGRAM is None:
        _PROGRAM = _build_program()
    res = bass_utils.run_bass_kernel_spmd(
        _PROGRAM, _in_maps(x, wq, wk, wv, wo),
        core_ids=list(range(8)), trace=TRACE)
    LAST_RESULTS = res
    out = np.zeros((B, S, D), np.float32)
    for c in range(8):
        out[c // NKV] += np.asarray(res.results[c]["out"], dtype=np.float32)
    return out



# revision 12
# speedup vs baseline: 1.1728x; 1.0212x over previous
"""Causal GQA attention layer (B=2, S=2048, D=2048, 16 Q heads / 4 KV heads,
interleaved RoPE, causal softmax, output projection) on 8 TRN2 NeuronCores.

Sharding: core c -> (batch b = c//4, kv-group g = c%4). Each core owns 4 Q
heads + 1 KV head (tensor parallel over heads) for one batch element (data
parallel over batch). wq/wk/wv are column-sharded, wo is row-sharded; each
core emits a partial [S, D] output and the host sums the 4 partials per batch.

Device dataflow per core (all matmuls bf16 with fp32 PSUM accumulation):
  - QT/KT/VT projections computed directly in transposed [hd, s] layout from
    host-pretransposed x^T and W^T (contiguous DMAs, no device transposes).
    V and K run d-major with 4 parallel PSUM accumulators so PE consumes xt
    tiles in DMA-arrival order (fills the initial load ramp).
  - RoPE applied in transposed layout: pair-swap via a permutation matmul on
    PE, then cos/sin elementwise on DVE with host-precomputed [hd, s] tables.
  - Scores computed transposed, ST[k, q] = K_rot^T Q_rot, chunked [128, 512]
    with causal chunk skipping and valid-width restriction on diagonal
    chunks; exp on ScalarE (PSUM->SBUF, bf16, scale 1/sqrt(hd), no max
    subtraction -- |scores|*scale <= ~6 so exp is safe); triangular mask on
    the diagonal 128x128 block via gpsimd.affine_select. Scores run LOOKAHEAD
    chunks ahead of the exp->PV consumers (software pipeline) so the in-order
    PE stream never parks on the exp semaphore.
  - attnT[hd, q] accumulated in PSUM over k-tiles with V as weights (no P
    transposes anywhere). Softmax denominators accumulate on DVE in bf16 and
    reduce with a single ones-matmul per head; normalization is deferred to a
    per-q-chunk epilogue (reciprocal_approx_fast + gpsimd.partition_broadcast
    + one DVE multiply) so PSUM banks recycle immediately.
  - Output projection (row-parallel) interleaved into the attention phase as
    PE filler work: after each head of q-chunk qc, one output s-tile of
    qc-1 is emitted; the last q-chunk's tiles run in a deep-buffered tail.
"""

import math

import numpy as np
import ml_dtypes

import concourse.bass as bass
import concourse.tile as tile
from concourse import bacc
from concourse import mybir
from concourse import bass_utils

BF = ml_dtypes.bfloat16

B, S, D = 2, 2048, 2048
NH, NKV, HD = 16, 4, 128
P = 128
DT = D // P            # 16 contraction tiles
SCH = 512              # free-dim chunk
NSC = S // SCH         # 4
NST = S // P           # 16
HPG = NH // NKV        # 4 q heads per core
ROPE_BASE = 10000.0
SCALE = 1.0 / math.sqrt(HD)

TRACE = False
LAST_RESULTS = None


NWARM = 26
LOOKP = 1              # lookahead in head-pair steps (psS pair tiles)


def _emit(nc, tc, aps):
    f32 = mybir.dt.float32
    bf16 = mybir.dt.bfloat16
    AF = mybir.ActivationFunctionType
    OP = mybir.AluOpType
    ctx_pools = []

    with tc.tile_pool(name="const", bufs=1) as cp:
        # ---- Phase 0: PE warmup ------------------------------------
        # Garbage matmuls on a zeroed tile, emitted before any
        # DMA-dependent work: the PE is busy from the post-barrier
        # instant (~3.5us), so the HAM clock-gate warms to 8/8 during the
        # DMA ramp and the first real matmuls run at full clock. wz lives
        # in the const pool: if its SBUF were recycled, the input DMAs
        # would inherit a WAR dependency on every warmup matmul.
        wz = cp.tile([P, SCH], bf16)
        nc.vector.memset(wz[:], 0.0)
        with tc.tile_pool(name="psW", bufs=1, space="PSUM") as psW:
            pw = psW.tile([P, SCH], f32, tag="w")
            for _ in range(NWARM):
                nc.tensor.matmul(pw[:], wz[:, 0:P], wz[:], start=True, stop=True)
        wvt = cp.tile([P, DT, HD], bf16)
        xt = cp.tile([P, DT, S], bf16)
        wkt = cp.tile([P, DT, HD], bf16)
        wqt = cp.tile([P, DT, HPG * HD], bf16)
        wot = cp.tile([P, HPG, D], bf16)
        cost = cp.tile([P, S], f32)
        sint = cp.tile([P, S], f32)
        perm = cp.tile([P, P], bf16)
        ident = cp.tile([P, P], bf16)
        ones1 = cp.tile([P, 1], bf16)
        # DMA triggers in consumption order (sc-major); HWDGE queue slots
        # round-robin in emission order across sync+scalar.
        engs = [nc.sync, nc.scalar]
        tq = []

        def dma(dst, src):
            tq.append((dst, src))

        dma(wvt[:], aps["wvt"][:])
        dma(xt[:, 0:4, 0:SCH], aps["xt"][:, 0:4, 0:SCH])
        dma(wqt[:, 0:4], aps["wqt"][:, 0:4])
        dma(wkt[:], aps["wkt"][:])
        dma(xt[:, 4:8, 0:SCH], aps["xt"][:, 4:8, 0:SCH])
        dma(wqt[:, 4:8], aps["wqt"][:, 4:8])
        dma(xt[:, 8:12, 0:SCH], aps["xt"][:, 8:12, 0:SCH])
        dma(wqt[:, 8:16], aps["wqt"][:, 8:16])
        dma(xt[:, 12:16, 0:SCH], aps["xt"][:, 12:16, 0:SCH])
        nc.gpsimd.dma_start(ident[:], aps["ident"][:])
        nc.gpsimd.dma_start(perm[:], aps["perm"][:])
        nc.gpsimd.dma_start(ones1[:], aps["ones1"][:])
        for sc in range(1, NSC):
            dma(xt[:, 0:8, sc * SCH:(sc + 1) * SCH],
                aps["xt"][:, 0:8, sc * SCH:(sc + 1) * SCH])
            dma(xt[:, 8:16, sc * SCH:(sc + 1) * SCH],
                aps["xt"][:, 8:16, sc * SCH:(sc + 1) * SCH])
            if sc == 1:
                dma(cost[:], aps["cost"][:])
                dma(sint[:], aps["sint"][:])
            if sc == 2:
                dma(wot[:, 0:2], aps["wot"][:, 0:2])
            if sc == 3:
                dma(wot[:, 2:4], aps["wot"][:, 2:4])
        for i, (dst, src_) in enumerate(tq):
            engs[i % 2].dma_start(dst, src_)

        # DVE warm-up: absorb the SWDGE semaphore tick for cost/sint into
        # DVE's vector clock so later tensor_tensor ops need only one fresh
        # wait (the TT encoding holds a single sync wait command).
        warm = cp.tile([1, 2], f32)
        nc.vector.tensor_copy(warm[:, 0:1], cost[0:1, 0:1])
        nc.vector.tensor_copy(warm[:, 1:2], sint[0:1, 0:1])
        # preload the Exp activation table during the DMA ramp; otherwise the
        # first score-exp in phase B pays a ~1.3us ACT_TABLE_LOAD inline
        nc.scalar.activation(warm[:, 0:1], warm[:, 0:1], AF.Exp, scale=1.0)
        # pre-swap the gpsimd ucode library during the DMA ramp so the
        # first affine_select/partition_broadcast in phase B doesn't pay
        # a ~5us mid-kernel library load
        warmg = cp.tile([P, 2], f32)
        nc.gpsimd.affine_select(
            out=warm[:, 0:1], in_=warm[:, 0:1], pattern=[[1, 1]],
            compare_op=OP.is_ge, fill=0.0, base=0, channel_multiplier=0)
        nc.gpsimd.partition_broadcast(warmg[:], warm[0:1, :])

        QROT = cp.tile([P, HPG, S], bf16)   # rotated Q^T per head
        KROT = cp.tile([P, S], bf16)        # rotated K^T
        VN = cp.tile([P, NST, HD], bf16)    # V natural [s_in, s_tile, hd]
        ATTN = cp.tile([P, HPG, S], bf16)   # normalized attn^T per head
        qt3 = cp.tile([P, 5, SCH], bf16)    # last-chunk K/Q bf16 staging

        # ---- Phase A: projections + RoPE + V transpose, sc-major --------
        with tc.tile_pool(name="psA", bufs=1, space="PSUM") as psA, \
             tc.tile_pool(name="sbA", bufs=2) as sbA:
            # V^T staging lives only in phase A; keeping it out of the
            # const pool frees 4KB/partition of SBUF for phase B
            VTS = sbA.tile([P, S], bf16, tag="vts", bufs=1)

            def rope_pe(qt, psw):
                nc.tensor.matmul(psw[:], perm[:], qt, start=True, stop=True)

            def rope_dve(qt, psw, sc, dst_ap):
                t1 = sbA.tile([P, SCH], f32, tag="t1")
                nc.vector.tensor_tensor(
                    t1[:], psw[:], sint[:, sc * SCH:(sc + 1) * SCH], OP.mult)
                t2 = sbA.tile([P, SCH], f32, tag="t2")
                nc.vector.tensor_tensor(
                    t2[:], qt, cost[:, sc * SCH:(sc + 1) * SCH], OP.mult)
                nc.vector.tensor_tensor(dst_ap, t1[:], t2[:], OP.add)

            def proj_chunk(sc, inject):
                vps = psA.tile([P, SCH], f32, tag="v", name=f"vps{sc}")
                kps = psA.tile([P, SCH], f32, tag="k", name=f"kps{sc}")
                qps = [psA.tile([P, SCH], f32, tag=f"q{h}", name=f"qps{sc}_{h}")
                       for h in range(HPG)]
                xsl = xt[:, :, sc * SCH:(sc + 1) * SCH]
                for dt_ in range(DT):
                    if dt_ in inject:
                        inject[dt_]()
                    nc.tensor.matmul(vps[:], wvt[:, dt_, :], xsl[:, dt_],
                                     start=(dt_ == 0), stop=(dt_ == DT - 1))
                    for h in range(HPG):
                        nc.tensor.matmul(
                            qps[h][:], wqt[:, dt_, h * HD:(h + 1) * HD],
                            xsl[:, dt_],
                            start=(dt_ == 0), stop=(dt_ == DT - 1))
                    nc.tensor.matmul(kps[:], wkt[:, dt_, :], xsl[:, dt_],
                                     start=(dt_ == 0), stop=(dt_ == DT - 1))
                nc.scalar.copy(VTS[:, sc * SCH:(sc + 1) * SCH], vps[:])
                if sc == NSC - 1:
                    qtk = qt3[:, 0, :]
                    qtq = [qt3[:, 1 + h, :] for h in range(HPG)]
                else:
                    qtk = sbA.tile([P, SCH], bf16, tag="qt", bufs=10,
                                   name=f"qtk{sc}")[:]
                    qtq = [sbA.tile([P, SCH], bf16, tag="qt", bufs=10,
                                    name=f"qtq{sc}_{h}")[:]
                           for h in range(HPG)]
                nc.scalar.copy(qtk, kps[:])
                for h in range(HPG):
                    nc.scalar.copy(qtq[h], qps[h][:])
                return qtk, qtq

            def deferred(sc, qtk, qtq):
                inj = {}

                def at(dt_, fn):
                    prev = inj.get(dt_)
                    if prev is None:
                        inj[dt_] = fn
                    else:
                        def both(prev=prev, fn=fn):
                            prev()
                            fn()
                        inj[dt_] = both

                for j in range(4):
                    def vtrans(j=j):
                        ki = 4 * sc + j
                        pst = psA.tile([P, P], bf16, tag="tr", name=f"pst{ki}")
                        nc.tensor.transpose(
                            pst[:], VTS[:, ki * P:(ki + 1) * P], ident[:])
                        nc.scalar.copy(VN[:, ki, :], pst[:])
                    at(1 + j, vtrans)

                def ropek():
                    psw = psA.tile([P, SCH], f32, tag="w", name=f"pswk{sc}")
                    rope_pe(qtk, psw)
                    rope_dve(qtk, psw, sc, KROT[:, sc * SCH:(sc + 1) * SCH])
                at(6, ropek)
                for h in range(HPG):
                    def ropeq(h=h):
                        psw = psA.tile([P, SCH], f32, tag="w",
                                       name=f"pswq{sc}_{h}")
                        rope_pe(qtq[h], psw)
                        rope_dve(qtq[h], psw, sc,
                                 QROT[:, h, sc * SCH:(sc + 1) * SCH])
                    at(8 + 2 * h, ropeq)
                return inj

            prev = None
            for sc in range(NSC):
                inj = {} if prev is None else deferred(sc - 1, *prev)
                prev = proj_chunk(sc, inj)
            for j in range(4):
                ki = 4 * (NSC - 1) + j
                pst = psA.tile([P, P], bf16, tag="tr", name=f"pst{ki}")
                nc.tensor.transpose(
                    pst[:], VTS[:, ki * P:(ki + 1) * P], ident[:])
                nc.scalar.copy(VN[:, ki, :], pst[:])

        # ---- Phase B+C: attention (software-pipelined) with the output
        # projection interleaved to fill PE bubbles ----------------------
        # Loop qc outer / head-PAIR inner: heads (0,1) then (2,3) share a
        # k-chunk step. The two heads' score matmuls land in the two banks
        # of one PSUM pair tile, so ONE exp ACTIVATE covers both ([128,1024]
        # crossing the bank boundary) and ONE DVE add accumulates both
        # running sums -- halving the per-op fixed costs on the scalar and
        # vector engines, which otherwise pace qc=0/qc=3. After each pair
        # step, one output-projection s-tile of qc-1 is emitted as PE
        # filler work.
        with tc.tile_pool(name="psS", bufs=LOOKP + 1, space="PSUM") as psS, \
             tc.tile_pool(name="psAV", bufs=2, space="PSUM") as psAV, \
             tc.tile_pool(name="psDN", bufs=1, space="PSUM") as psDN, \
             tc.tile_pool(name="psO", bufs=1, space="PSUM") as psO, \
             tc.tile_pool(name="sbB", bufs=3) as sbB, \
             tc.tile_pool(name="sbB2", bufs=5) as sbB2, \
             tc.tile_pool(name="sbN", bufs=2) as sbN, \
             tc.tile_pool(name="sbC", bufs=4) as sbC:

            def emit_c_group(st, oc, use_act=False):
                po = psO.tile([P, SCH], f32, tag="o")
                for ct in range(HPG):
                    nc.tensor.matmul(
                        po[:], ATTN[:, ct, st * P:(st + 1) * P],
                        wot[:, ct, oc * SCH:(oc + 1) * SCH],
                        start=(ct == 0), stop=(ct == HPG - 1))
                ob = sbC.tile([P, SCH], bf16, tag="ob")
                if use_act:
                    nc.scalar.copy(ob[:], po[:])
                else:
                    nc.vector.tensor_copy(ob[:], po[:])
                nc.sync.dma_start(
                    aps["out"][st * P:(st + 1) * P, oc * SCH:(oc + 1) * SCH], ob[:])

            sc3 = NSC - 1
            rope3 = list(range(5))
            groups = []
            epi_q = []      # deferred per-head epilogues, popped 1/step
            carry_epi = []  # h=3 epilogue carried into the next qc's loop

            def make_epi(qc, h, pbs, j, aun):
                def epi():
                    pdn = psDN.tile([1, SCH], f32, tag="dn")
                    nc.tensor.matmul(pdn[:], ones1[:], pbs[:, j, :],
                                     start=True, stop=True)
                    rc = sbN.tile([1, SCH], f32, tag="rc")
                    nc.vector.reciprocal_approx_fast(out=rc[:], in_=pdn[:])
                    bc = sbN.tile([P, SCH], f32, tag="bc")
                    nc.gpsimd.partition_broadcast(bc[:], rc[:])
                    nc.vector.tensor_tensor(
                        ATTN[:, h, qc * SCH:(qc + 1) * SCH], aun[:], bc[:],
                        OP.mult)
                return epi

            def emit_rope3():
                j = rope3.pop(0)
                psw = psDN.tile([P, SCH], f32, tag="dn", name=f"psw3_{j}")
                nc.tensor.matmul(psw[:], perm[:], qt3[:, j, :],
                                 start=True, stop=True)
                t1 = sbN.tile([P, SCH], bf16, tag="t1", name=f"t13_{j}")
                nc.vector.tensor_tensor(
                    t1[:], psw[:], sint[:, sc3 * SCH:(sc3 + 1) * SCH], OP.mult)
                t2 = sbN.tile([P, SCH], bf16, tag="t2", name=f"t23_{j}")
                nc.vector.tensor_tensor(
                    t2[:], qt3[:, j, :], cost[:, sc3 * SCH:(sc3 + 1) * SCH],
                    OP.mult)
                dst = KROT[:, sc3 * SCH:(sc3 + 1) * SCH] if j == 0 \
                    else QROT[:, j - 1, sc3 * SCH:(sc3 + 1) * SCH]
                nc.vector.tensor_tensor(dst, t1[:], t2[:], OP.add)

            # rope3 PE+DVE work is spread over qc=0..2 (its outputs are
            # first consumed by qc=3); qc=0's DVE would otherwise pace it
            rope3_sched = {0: (2,), 1: (2, 6), 2: (2, 6)}
            for qc in range(NSC):
                nki = 4 * qc + 4
                pairs = [(hp, ki) for hp in range(2) for ki in range(nki)]
                slots = {}
                accs = {}
                psums = {}

                def emit_score(c, qc=qc):
                    hp, ki = c
                    off = max(0, (ki - 4 * qc) * P)
                    n = SCH - off
                    psp = psS.tile([P, 2, SCH], f32, tag="s")
                    for j in range(2):
                        h = 2 * hp + j
                        nc.tensor.matmul(
                            psp[:, j, :n], KROT[:, ki * P:(ki + 1) * P],
                            QROT[:, h, qc * SCH + off: qc * SCH + off + n],
                            start=True, stop=True)
                    slots[c] = (psp, off, n)

                def emit_tail(c, qc=qc, nki=nki):
                    hp, ki = c
                    psp, off, n = slots.pop(c)
                    if ki == 0:
                        psums[hp] = sbB2.tile([P, 2, SCH], bf16, tag="ps",
                                              name=f"pbs{qc}_{hp}")
                        for j in range(2):
                            h = 2 * hp + j
                            accs[h] = (
                                psAV.tile([P, SCH], f32, tag="av",
                                          name=f"pav{qc}_{h}"),
                                sbB2.tile([P, SCH], bf16, tag="au",
                                          name=f"aun{qc}_{h}"))
                    pbsum = psums[hp]
                    # ki==0: exp writes the running-sum tile directly (the
                    # first summand IS the exp output); PV reads it as moving
                    pb = pbsum if ki == 0 else sbB.tile([P, 2, SCH], bf16,
                                                        tag="p")
                    nc.scalar.activation(pb[:, :, :n], psp[:, :, :n], AF.Exp,
                                         scale=SCALE)
                    if ki >= 4 * qc:
                        for j in range(2):
                            nc.gpsimd.affine_select(
                                out=pb[:, j, :P], in_=pb[:, j, :P],
                                pattern=[[1, P]], compare_op=OP.is_ge,
                                fill=0.0, base=0, channel_multiplier=-1)
                    for j in range(2):
                        h = 2 * hp + j
                        nc.tensor.matmul(
                            accs[h][0][:, off:], VN[:, ki, :], pb[:, j, :n],
                            start=(ki == 0), stop=(ki == nki - 1))
                    if ki > 0:
                        nc.vector.tensor_tensor(
                            pbsum[:, :, off:], pbsum[:, :, off:],
                            pb[:, :, :n], OP.add)
                    if ki == nki - 1:
                        # unnormalized attn out of PSUM fast (frees the
                        # banks); normalization runs as deferred epilogues
                        for j in range(2):
                            h = 2 * hp + j
                            pav, aun = accs[h]
                            nc.scalar.copy(aun[:], pav[:])
                            if hp == 0:
                                # pair (0,1) finishes mid-loop: normalize here
                                epi_q.append(make_epi(qc, h, pbsum, j, aun))
                            elif qc < NSC - 1:
                                # pair (2,3) of qc<3: normalize early in
                                # qc+1's loop so the PE never parks on the
                                # exp->pbsum chain at the qc boundary
                                carry_epi.append(make_epi(qc, h, pbsum, j, aun))
                        if qc > 0:
                            for j in range(2):
                                st = (qc - 1) * 4 + 2 * hp + j
                                for oc in range(NSC):
                                    groups.append((st, oc))

                epi_q.extend(carry_epi)
                carry_epi = []
                for i in range(len(pairs) + LOOKP):
                    if i < len(pairs):
                        emit_score(pairs[i])
                    if i >= LOOKP:
                        emit_tail(pairs[i - LOOKP])
                        if groups:
                            st_, oc_ = groups.pop(0)
                            emit_c_group(st_, oc_, use_act=(oc_ == 3))
                        if epi_q:
                            epi_q.pop(0)()
                        if (i - LOOKP) in rope3_sched.get(qc, ()) and rope3:
                            emit_rope3()

                while groups:
                    st_, oc_ = groups.pop(0)
                    emit_c_group(st_, oc_, use_act=False)
                while epi_q:
                    epi_q.pop(0)()
                if qc == NSC - 1:
                    # qc3 pair (2,3): the only epilogues on the tail critical
                    # path; the flush groups above give the PE cover while the
                    # exp->pbsum chain drains. h=3 normalizes in 4 per-st
                    # pieces so the first tail out-proj tile starts after a
                    # [128,128] multiply, not the full 512-wide one.
                    make_epi(qc, 2, psums[1], 0, accs[2][1])()
                    aun = accs[HPG - 1][1]
                    pdn = psDN.tile([1, SCH], f32, tag="dn")
                    nc.tensor.matmul(pdn[:], ones1[:], psums[1][:, 1, :],
                                     start=True, stop=True)
                    rc = sbN.tile([1, SCH], f32, tag="rc")
                    nc.vector.reciprocal_approx_fast(out=rc[:], in_=pdn[:])
                    bc = sbN.tile([P, SCH], f32, tag="bc")
                    nc.gpsimd.partition_broadcast(bc[:], rc[:])
                    for stp in range(4):
                        nc.vector.tensor_tensor(
                            ATTN[:, HPG - 1,
                                 qc * SCH + stp * P: qc * SCH + (stp + 1) * P],
                            aun[:, stp * P:(stp + 1) * P],
                            bc[:, stp * P:(stp + 1) * P], OP.mult)

        # ---- tail: last q-chunk's output tiles with deep PSUM buffering.
        # DMA fires per [128,1024] half as soon as its two oc-chunks are
        # copied, round-robin across sync/scalar/gpsimd queues, so the last
        # transfer is small and the drain overlaps the remaining compute.
        with tc.tile_pool(name="psO2", bufs=6, space="PSUM") as psO2, \
             tc.tile_pool(name="sbC2", bufs=3) as sbC2:
            dma_rr = [nc.sync, nc.scalar, nc.gpsimd]
            nrr = 0
            for st in range(3 * 4, NST):
                ob = sbC2.tile([P, D], bf16, tag="ob2", name=f"obt{st}")
                for oc in range(NSC):
                    po = psO2.tile([P, SCH], f32, tag="o2")
                    for ct in range(HPG):
                        nc.tensor.matmul(
                            po[:], ATTN[:, ct, st * P:(st + 1) * P],
                            wot[:, ct, oc * SCH:(oc + 1) * SCH],
                            start=(ct == 0), stop=(ct == HPG - 1))
                    if oc % 2 == 0:
                        nc.scalar.copy(ob[:, oc * SCH:(oc + 1) * SCH], po[:])
                    else:
                        nc.vector.tensor_copy(ob[:, oc * SCH:(oc + 1) * SCH], po[:])
                    if oc % 2 == 1:
                        half = (oc // 2) * 2 * SCH
                        dma_rr[nrr % 3].dma_start(
                            aps["out"][st * P:(st + 1) * P, half:half + 2 * SCH],
                            ob[:, half:half + 2 * SCH])
                        nrr += 1



def _build_program():
    f32 = mybir.dt.float32
    bf16 = mybir.dt.bfloat16
    nc = bacc.Bacc("TRN2", debug=False, target_bir_lowering=False)
    aps = {
        "xt": nc.dram_tensor("xt", [P, DT, S], bf16, kind="ExternalInput").ap(),
        "wqt": nc.dram_tensor("wqt", [P, DT, HPG * HD], bf16, kind="ExternalInput").ap(),
        "wkt": nc.dram_tensor("wkt", [P, DT, HD], bf16, kind="ExternalInput").ap(),
        "wvt": nc.dram_tensor("wvt", [P, DT, HD], bf16, kind="ExternalInput").ap(),
        "wot": nc.dram_tensor("wot", [P, HPG, D], bf16, kind="ExternalInput").ap(),
        "cost": nc.dram_tensor("cost", [P, S], f32, kind="ExternalInput").ap(),
        "sint": nc.dram_tensor("sint", [P, S], f32, kind="ExternalInput").ap(),
        "perm": nc.dram_tensor("perm", [P, P], bf16, kind="ExternalInput").ap(),
        "ident": nc.dram_tensor("ident", [P, P], bf16, kind="ExternalInput").ap(),
        "ones1": nc.dram_tensor("ones1", [P, 1], bf16, kind="ExternalInput").ap(),
        "out": nc.dram_tensor("out", [S, D], bf16, kind="ExternalOutput").ap(),
    }
    with tile.TileContext(nc) as tc:
        _emit(nc, tc, aps)
    nc.compile()
    return nc


def _tables():
    theta = 1.0 / (ROPE_BASE ** (np.arange(0, HD, 2, dtype=np.float64) / HD))
    ang = np.outer(np.arange(S, dtype=np.float64), theta)      # [S, 64]
    cosT = np.repeat(np.cos(ang).T, 2, axis=0).astype(np.float32)  # [128, S]
    sinT = np.repeat(np.sin(ang).T, 2, axis=0)
    sign = np.where(np.arange(HD) % 2 == 0, -1.0, 1.0)[:, None]
    sinsT = (sinT * sign).astype(np.float32)
    perm = np.zeros((P, P), dtype=BF)
    idx = np.arange(P)
    perm[idx, idx ^ 1] = 1
    ident = np.eye(P, dtype=np.float32).astype(BF)
    ones1 = np.ones((P, 1), dtype=BF)
    return cosT, sinsT, perm, ident, ones1


def _in_maps(x, wq, wk, wv, wo):
    cosT, sinsT, perm, ident, ones1 = _tables()
    maps = []
    for c in range(8):
        b, g = divmod(c, NKV)
        xt = x[b].T.reshape(DT, P, S).transpose(1, 0, 2).astype(BF)
        wqg = wq[g * HPG * HD:(g + 1) * HPG * HD]
        wqt = wqg.T.reshape(DT, P, HPG * HD).transpose(1, 0, 2).astype(BF)
        wkt = wk[g * HD:(g + 1) * HD].T.reshape(DT, P, HD).transpose(1, 0, 2).astype(BF)
        wvt = wv[g * HD:(g + 1) * HD].T.reshape(DT, P, HD).transpose(1, 0, 2).astype(BF)
        wog = wo[:, g * HPG * HD:(g + 1) * HPG * HD]
        wot = wog.T.reshape(HPG, P, D).transpose(1, 0, 2).astype(BF)
        maps.append({
            "xt": np.ascontiguousarray(xt),
            "wqt": np.ascontiguousarray(wqt),
            "wkt": np.ascontiguousarray(wkt),
            "wvt": np.ascontiguousarray(wvt),
            "wot": np.ascontiguousarray(wot),
            "cost": cosT, "sint": sinsT,
            "perm": perm, "ident": ident, "ones1": ones1,
        })
    return maps


_PROGRAM = None


def kernel(x, wq, wk, wv, wo):
    global _PROGRAM, LAST_RESULTS
    x = np.asarray(x, dtype=np.float32)
    wq = np.asarray(wq, dtype=np.float32)
    wk = np.asarray(wk, dtype=np.float32)
    wv = np.asarray(wv, dtype=np.float32)
    wo = np.asarray(wo, dtype=np.float32)
    if _PROGRAM is None:
        _PROGRAM = _build_program()
    res = bass_utils.run_bass_kernel_spmd(
        _PROGRAM, _in_maps(x, wq, wk, wv, wo),
        core_ids=list(range(8)), trace=TRACE)
    LAST_RESULTS = res
    out = np.zeros((B, S, D), np.float32)
    for c in range(8):
        out[c // NKV] += np.asarray(res.results[c]["out"], dtype=np.float32)
    return out

